# revision 7
# baseline (speedup 1.0000x reference)
"""Trainium2 Bass kernel for nn_Autocorrelation (B=16, L=1024, D=512, H=8, dh=64).

Self-contained: kernel(**inputs) -> np.ndarray [16, 1024, 512] float32.

Redesign over the F=640 baseline (see git-less history in test logs):
- F=512: real-input spectrum f in [1, 512]. The f=0 (DC) term is dropped:
  for stage 1 it shifts every corr lag by a per-row constant, which top-k
  selection and softmax are both invariant to; for stage 2 it is a per-row
  constant (rowsum(v') * sum(s) = rowsum(v')) added back via the final
  PSUM->SBUF copy bias. alpha (2, except Nyquist 1) is folded into the
  host-precomputed inverse DFT matrices Ci/Si.
- Bias-free projections: a time-constant bias only feeds f=0, so q/k/v are
  projected without bq entirely; bq enters only the stage-2 DC correction.
  1/L is folded into the q and v projections (Wl = Wq/L).
- Proj psum is copied out on the Scalar engine (v's copies also emit
  accum_out row-sums for the DC correction); rows/spectra copies ride the
  GpSimd engine; pointwise complex products read PSUM directly on DVE.
- top-13 in f16 (max8 x2 + match_replace); masked softmax fused into one
  scalar_tensor_tensor with accum_out for the normalizer.
- Output is written as [row=(b,dh), tau] f16; the host transposes/tiles.
"""

import threading

import numpy as np

L = 1024
D = 512
DH = 64
BLOC = 2          # batches per core
B = 16
H = 8
KTOP = 13
NCORES = 8
F = 512
FC = 4            # inverse-side f chunks of 128
JC = 8            # time chunks of 128


def _build_nc(cfg=None):
    from contextlib import ExitStack

    import concourse.bass as bass
    import concourse.mybir as mybir
    import concourse.tile as tile
    from concourse import bacc
    from concourse.masks import make_identity

    f32 = mybir.dt.float32
    f16 = mybir.dt.float16
    AF = mybir.ActivationFunctionType
    ALU = mybir.AluOpType

    nc = bacc.Bacc("TRN2", target_bir_lowering=False, debug=False, num_devices=NCORES)

    Qf = nc.declare_dram_parameter("Qf", [BLOC, D, L], f16, isOutput=False)
    Kf = nc.declare_dram_parameter("Kf", [BLOC, D, L], f16, isOutput=False)
    Vf = nc.declare_dram_parameter("Vf", [BLOC, D, L], f16, isOutput=False)
    Wlf = nc.declare_dram_parameter("Wlf", [D, DH], f16, isOutput=False)   # Wq/L
    Wuf = nc.declare_dram_parameter("Wuf", [D, DH], f16, isOutput=False)   # Wq
    Bcf = nc.declare_dram_parameter("Bcf", [128], f32, isOutput=False)     # tile(bq,2)
    Cff = nc.declare_dram_parameter("Cff", [L, F], f16, isOutput=False)
    Sff = nc.declare_dram_parameter("Sff", [L, F], f16, isOutput=False)
    Cif = nc.declare_dram_parameter("Cif", [F, L], f16, isOutput=False)    # alpha-folded
    Sif = nc.declare_dram_parameter("Sif", [F, L], f16, isOutput=False)
    outd = nc.declare_dram_parameter("out", [128, L], f16, isOutput=True)

    with tile.TileContext(nc) as tc, ExitStack() as ctx:
        consts = ctx.enter_context(tc.tile_pool(name="consts", bufs=1))
        inp = ctx.enter_context(tc.tile_pool(name="inp", bufs=1))
        pjp = ctx.enter_context(tc.tile_pool(name="pjp", bufs=2))
        rowsp = ctx.enter_context(tc.tile_pool(name="rowsp", bufs=1))
        specp = ctx.enter_context(tc.tile_pool(name="specp", bufs=1))
        rowbig = ctx.enter_context(tc.tile_pool(name="rowbig", bufs=1))
        small = ctx.enter_context(tc.tile_pool(name="small", bufs=1))
        ps_pj = ctx.enter_context(tc.tile_pool(name="ps_pj", bufs=2, space="PSUM"))
        ps_tr = ctx.enter_context(tc.tile_pool(name="ps_tr", bufs=2, space="PSUM"))
        ps_sp = ctx.enter_context(tc.tile_pool(name="ps_sp", bufs=4, space="PSUM"))

        def as_col(ap):
            return bass.AP(tensor=ap.tensor, offset=ap.offset,
                           ap=list(ap.ap) + [[0, 1]])

        # ---- DMA issue: inputs on sync, Cf/Sf on scalar, rest on gpsimd ----
        thQ = inp.tile([128, 4, BLOC, L], f16, name="thQ")
        thK = inp.tile([128, 4, BLOC, L], f16, name="thK")
        thV = inp.tile([128, 4, BLOC, L], f16, name="thV")
        for b in range(BLOC):
            nc.sync.dma_start(out=thQ[:, :, b, :],
                              in_=Qf[b].rearrange("(c p) l -> p c l", p=128))
        for b in range(BLOC):
            nc.sync.dma_start(out=thK[:, :, b, :],
                              in_=Kf[b].rearrange("(c p) l -> p c l", p=128))
        for b in range(BLOC):
            nc.sync.dma_start(out=thV[:, :, b, :],
                              in_=Vf[b].rearrange("(c p) l -> p c l", p=128))

        Cf_sb = consts.tile([128, JC, F], f16, name="Cf_sb")
        Sf_sb = consts.tile([128, JC, F], f16, name="Sf_sb")
        nc.scalar.dma_start(out=Cf_sb, in_=Cff.rearrange("(a p) x -> p a x", p=128))
        nc.scalar.dma_start(out=Sf_sb, in_=Sff.rearrange("(a p) x -> p a x", p=128))

        Wl_sb = consts.tile([128, 4, DH], f16, name="Wl_sb")
        Wu_sb = consts.tile([128, 4, DH], f16, name="Wu_sb")
        nc.gpsimd.dma_start(out=Wl_sb, in_=Wlf.rearrange("(c p) h -> p c h", p=128))
        nc.gpsimd.dma_start(out=Wu_sb, in_=Wuf.rearrange("(c p) h -> p c h", p=128))
        bcol = consts.tile([128, 1], f32, name="bcol")
        nc.gpsimd.dma_start(out=bcol, in_=as_col(Bcf[:]))
        Ci_sb = consts.tile([128, FC, L], f16, name="Ci_sb")
        Si_sb = consts.tile([128, FC, L], f16, name="Si_sb")
        nc.gpsimd.dma_start(out=Ci_sb, in_=Cif.rearrange("(a p) x -> p a x", p=128))
        nc.gpsimd.dma_start(out=Si_sb, in_=Sif.rearrange("(a p) x -> p a x", p=128))

        # scalar-engine act table warm-up (1.3us, off critical path)
        warm = small.tile([128, 1], f32, name="warm")
        nc.gpsimd.memset(warm, 0.0)
        nc.scalar.activation(warm, warm, AF.Exp, bias=0.0, scale=1.0)

        identh = consts.tile([128, 128], f16, name="identh")
        make_identity(nc, identh)

        # ---- projection (W stationary) + transpose to rows[j, jc, r] ----
        rows_q = rowsp.tile([128, JC, 128], f16, name="rows_q")
        rows_k = rowsp.tile([128, JC, 128], f16, name="rows_k")
        rows_v = rowsp.tile([128, JC, 128], f16, name="rows_v")
        vsa = small.tile([DH, 4], f32, name="vsa")

        def project(th, Wsb, rows_dst, is_v):
            for b in range(BLOC):
                projT = pjp.tile([DH, L], f16, tag="projT")
                for hh in range(2):
                    pj = ps_pj.tile([DH, 512], f32, tag="pj")
                    for dc in range(4):
                        nc.tensor.matmul(pj, lhsT=Wsb[:, dc, :],
                                         rhs=th[:, dc, b, hh * 512:(hh + 1) * 512],
                                         start=dc == 0, stop=dc == 3)
                    acc = vsa[:, 2 * b + hh:2 * b + hh + 1] if is_v else None
                    nc.scalar.activation(projT[:, hh * 512:(hh + 1) * 512], pj,
                                         AF.Copy, bias=0.0, scale=1.0,
                                         accum_out=acc)
                tp = ps_tr.tile([128, JC, 128], f16, tag="tr")
                for lt in range(JC):
                    nc.tensor.transpose(tp[:, lt, 0:DH],
                                        projT[:, lt * 128:(lt + 1) * 128],
                                        identh[:DH, :DH])
                nc.scalar.activation(rows_dst[:, :, DH * b:DH * (b + 1)],
                                     tp[:, :, 0:DH], AF.Copy, bias=0.0, scale=1.0)

        project(thQ, Wl_sb, rows_q, False)
        project(thK, Wu_sb, rows_k, False)

        # ---- forward DFT (rows stationary): spectra [r, f] in PSUM ----
        def fwd(rows_src):
            psr = ps_sp.tile([128, F], f32, tag="spec")
            psi = ps_sp.tile([128, F], f32, tag="spec")
            for jc in range(JC):
                st, sp = jc == 0, jc == JC - 1
                nc.tensor.matmul(psr, lhsT=rows_src[:, jc, :], rhs=Cf_sb[:, jc, :],
                                 start=st, stop=sp)
                nc.tensor.matmul(psi, lhsT=rows_src[:, jc, :], rhs=Sf_sb[:, jc, :],
                                 start=st, stop=sp)
            return psr, psi

        QRp, QIp = fwd(rows_q)
        KRp, KIp = fwd(rows_k)

        # ---- pointwise X = Qhat * conj(Khat) (one PSUM operand max/op) ----
        KR16 = specp.tile([128, F], f16, name="KR16")
        KI16 = specp.tile([128, F], f16, name="KI16")
        nc.scalar.activation(KR16, KRp, AF.Copy, bias=0.0, scale=1.0)
        nc.scalar.activation(KI16, KIp, AF.Copy, bias=0.0, scale=1.0)
        t1 = rowbig.tile([128, F], f16, name="t1")
        t2 = rowbig.tile([128, F], f16, name="t2")
        XR = specp.tile([128, F], f16, name="XR")
        XI = specp.tile([128, F], f16, name="XI")
        nc.vector.tensor_mul(t1, QRp, KR16)
        nc.vector.tensor_mul(t2, QIp, KI16)
        nc.vector.tensor_add(XR, t1, t2)
        nc.vector.tensor_mul(t1, QIp, KR16)
        nc.vector.tensor_mul(t2, QRp, KI16)
        nc.vector.tensor_sub(XI, t1, t2)

        # project V now; fwd(v) fills the PE while DVE runs topk later
        project(thV, Wl_sb, rows_v, True)

        # ---- chunk-transpose spectra to [f, fc, r] for the inverse ----
        def to_chunks(src, nch, copy_eng):
            tp = ps_tr.tile([128, JC, 128], f16, tag="tr")
            for fc in range(nch):
                nc.tensor.transpose(tp[:, fc, :], src[:, fc * 128:(fc + 1) * 128],
                                    identh)
            dst = specp.tile([128, nch, 128], f16, name=f"T{src.tensor.name}")
            if copy_eng is nc.scalar:
                nc.scalar.activation(dst, tp[:, 0:nch, :], AF.Copy,
                                     bias=0.0, scale=1.0)
            else:
                copy_eng.tensor_copy(dst, tp[:, 0:nch, :])
            return dst

        XRT = to_chunks(XR, FC, nc.vector)
        XIT = to_chunks(XI, FC, nc.scalar)

        # ---- inverse DFT 1: corr[r, tau] ----
        corr16 = rowbig.tile([128, L], f16, name="corr16")
        for hh in range(2):
            pc = ps_sp.tile([128, 512], f32, tag="spec")
            sl = slice(hh * 512, (hh + 1) * 512)
            for fc in range(FC):
                nc.tensor.matmul(pc, lhsT=XRT[:, fc, :], rhs=Ci_sb[:, fc, sl],
                                 start=fc == 0, stop=False)
                nc.tensor.matmul(pc, lhsT=XIT[:, fc, :], rhs=Si_sb[:, fc, sl],
                                 start=False, stop=fc == FC - 1)
            nc.scalar.activation(corr16[:, sl], pc, AF.Copy, bias=0.0, scale=1.0)

        VRp, VIp = fwd(rows_v)
        VR16 = specp.tile([128, F], f16, name="VR16")
        VI16 = specp.tile([128, F], f16, name="VI16")
        nc.scalar.activation(VR16, VRp, AF.Copy, bias=0.0, scale=1.0)
        nc.scalar.activation(VI16, VIp, AF.Copy, bias=0.0, scale=1.0)

        # ---- top-13 via masked softmax (f16, no indices) ----
        vals16 = small.tile([128, 16], f16, name="vals16")
        corr2 = rowbig.tile([128, L], f16, name="corr2")
        nc.vector.max(out=vals16[:, 0:8], in_=corr16)
        nc.vector.match_replace(out=corr2, in_to_replace=vals16[:, 0:8],
                                in_values=corr16, imm_value=-60000.0)
        nc.vector.max(out=vals16[:, 8:16], in_=corr2)
        negm = small.tile([128, 1], f32, name="negm")
        nc.vector.tensor_scalar_mul(negm, vals16[:, 0:1], -1.0)
        ecorr = rowbig.tile([128, L], f16, name="ecorr")
        nc.scalar.activation(ecorr, corr16, AF.Exp, bias=negm, scale=1.0)
        em = rowbig.tile([128, L], f16, name="em")
        ssum = small.tile([128, 1], f32, name="ssum")
        nc.vector.scalar_tensor_tensor(em, in0=corr16, scalar=vals16[:, 12:13],
                                       in1=ecorr, op0=ALU.is_ge, op1=ALU.mult,
                                       accum_out=ssum)
        rs = small.tile([128, 1], f32, name="rs")
        nc.vector.reciprocal(rs, ssum)
        s16 = rowbig.tile([128, L], f16, name="s16")
        nc.vector.tensor_scalar(s16, em, scalar1=rs, scalar2=None, op0=ALU.mult)

        # DC correction column: rowsum(v') + bq
        vsum = small.tile([128, 1], f32, name="vsum")
        nc.vector.tensor_add(vsum[0:DH, :], vsa[:, 0:1], vsa[:, 1:2])
        nc.vector.tensor_add(vsum[DH:128, :], vsa[:, 2:3], vsa[:, 3:4])
        corrcol = small.tile([128, 1], f32, name="corrcol")
        nc.vector.tensor_add(corrcol, vsum, bcol)

        # ---- stage 2: s transpose, fwd(s), Y = Vhat * conj(Shat) ----
        sT = to_chunks(s16, JC, nc.vector)
        SRp, SIp = fwd(sT)

        u1 = rowbig.tile([128, F], f16, name="u1")
        u2 = rowbig.tile([128, F], f16, name="u2")
        YR = specp.tile([128, F], f16, name="YR")
        YI = specp.tile([128, F], f16, name="YI")
        nc.vector.tensor_mul(u1, VR16, SRp)
        nc.vector.tensor_mul(u2, VI16, SIp)
        nc.vector.tensor_add(YR, u1, u2)
        nc.vector.tensor_mul(u1, VI16, SRp)
        nc.vector.tensor_mul(u2, VR16, SIp)
        nc.vector.tensor_sub(YI, u1, u2)

        YRT = to_chunks(YR, FC, nc.vector)
        YIT = to_chunks(YI, FC, nc.scalar)

        # ---- inverse DFT 2 + DC bias + out DMA ----
        out16 = rowbig.tile([128, L], f16, name="out16")
        for hh in range(2):
            po = ps_sp.tile([128, 512], f32, tag="spec")
            sl = slice(hh * 512, (hh + 1) * 512)
            for fc in range(FC):
                nc.tensor.matmul(po, lhsT=YRT[:, fc, :], rhs=Ci_sb[:, fc, sl],
                                 start=fc == 0, stop=False)
                nc.tensor.matmul(po, lhsT=YIT[:, fc, :], rhs=Si_sb[:, fc, sl],
                                 start=False, stop=fc == FC - 1)
            nc.vector.tensor_scalar(out16[:, sl], po, scalar1=corrcol, scalar2=None,
                                    op0=ALU.add)
            nc.sync.dma_start(out=outd[:, sl], in_=out16[:, sl])

    nc.compile()
    return nc


_cache = threading.Lock(), {}


def _get_nc():
    lock, store = _cache
    with lock:
        if "nc" not in store:
            store["nc"] = _build_nc()
        return store["nc"]


def _make_consts():
    j = np.arange(L, dtype=np.float64)
    fv = np.arange(1, F + 1, dtype=np.float64)
    ang = 2.0 * np.pi * np.outer(j, fv) / L
    Cf = np.cos(ang).astype(np.float16)
    Sf = (-np.sin(ang)).astype(np.float16)
    alpha = np.full((F, 1), 2.0)
    alpha[-1, 0] = 1.0
    angi = 2.0 * np.pi * np.outer(fv, j) / L
    Ci = (alpha * np.cos(angi)).astype(np.float16)
    Si = (alpha * -np.sin(angi)).astype(np.float16)
    return Cf, Sf, Ci, Si


def _make_in_maps(Q, K, V, Wq, bq):
    Q = np.ascontiguousarray(Q, np.float32)
    K = np.ascontiguousarray(K, np.float32)
    V = np.ascontiguousarray(V, np.float32)
    Wq = np.ascontiguousarray(Wq, np.float32)
    bq = np.ascontiguousarray(bq, np.float32)

    def tr16(x):
        return np.ascontiguousarray(np.swapaxes(x, 1, 2).astype(np.float16))

    Qt, Kt, Vt = tr16(Q), tr16(K), tr16(V)
    Cf, Sf, Ci, Si = _make_consts()
    Wl16 = (Wq / L).astype(np.float16)
    Wu16 = Wq.astype(np.float16)
    bc = np.concatenate([bq, bq]).astype(np.float32)
    in_maps = []
    for c in range(NCORES):
        sl = slice(BLOC * c, BLOC * (c + 1))
        in_maps.append(
            {
                "Qf": Qt[sl], "Kf": Kt[sl], "Vf": Vt[sl],
                "Wlf": Wl16, "Wuf": Wu16, "Bcf": bc,
                "Cff": Cf, "Sff": Sf, "Cif": Ci, "Sif": Si,
            }
        )
    return in_maps


def _assemble(outs):
    # outs[c]: [128, L] f16, rows r = 64*b + dh for batches (2c, 2c+1)
    parts = []
    for c in range(NCORES):
        r = outs[c].reshape(BLOC, DH, L)          # [b, dh, tau]
        parts.append(np.swapaxes(r, 1, 2))        # [b, tau, dh]
    compact = np.concatenate(parts, axis=0).astype(np.float32)
    return np.tile(compact, (1, 1, H))


def kernel(Q, K, V, Wq, bq):
    from concourse.bass_utils import run_bass_kernel_spmd

    nc = _get_nc()
    in_maps = _make_in_maps(Q, K, V, Wq, bq)
    res = run_bass_kernel_spmd(nc, in_maps, list(range(NCORES)))
    return _assemble([res.results[i]["out"] for i in range(NCORES)])
